# revision 30
# baseline (speedup 1.0000x reference)
"""NonLocalBlock (self-attention over 64x64 image, C=256, D=32) on 8 trn2 cores.

Sharding: data-parallel over B=4 batches x 2-way split of the attention
rows (the `n` axis of beta[n, m]) => 8 cores, each computing a [2048, 256]
slice of the output. Each core receives its batch image pre-transposed
(and fp16-cast) by the host, rolled so its own 2048 rows come first,
plus its own half in natural layout for the residual. The host also
pre-casts the 1x1-conv weights (packed into one tensor) and folds gamma
into Wv (packed with the transpose identity).

Device math (per core, n = its 2048 key rows, m = all 4096 queries):
  logits[m, n] = q_m . k_n               PE, fp16, 2x [128m x 512n] matmuls
  E[m, n] = exp(logits)                  ACT exact exp | DVE exp2 bit-trick
                                         (int16(l*128*log2e + 16250.875)
                                          bitcast to bf16, trunc-fitted),
                                         [128, 1024] tiles from a shared
                                         3-deep PSUM rotation
  o[n, :] = sum_m E[m, n] v_aug[m, :]    PE, E stationary (33 cols/matmul),
                                         per-supergroup lag 4/6 behind exp;
                                         8 chains share one PSUM zero-region
  obar = o[:, 0:32] / o[:, 32]           DVE reciprocal; scale fused into the
                                         final scalar_tensor_tensor / ACT
                                         activation-scale epilogue
  oT = transpose(obar)                   PE (identity-moving), paired tiles
  out[n, :] = oT.T @ (gamma Wv) + x      PE matmul pairs + {ACT scale-copy +
                                         Pool add | DVE fused mul-add},
                                         batched [128, 4, 256] DMA out

Engine layout: exp tiles [128,1024] rotate through one 3-buf PSUM pool
(12KB); stages (proj chunks/transposes/final pairs) use a 1-buf 2KB pool
spaced >=2 steps apart; o accumulator 1 bank. ACT:DVE exp split ~35:29.
"""

from contextlib import ExitStack

import ml_dtypes
import numpy as np

import concourse.bass as bass
import concourse.tile as tile
from concourse import bacc, mybir
from concourse.bass_utils import run_bass_kernel_spmd

B, H, W, C = 4, 64, 64, 256
N = H * W            # 4096 pixels per image
D = 32               # reduced channel dim
NH = N // 2          # key rows owned by each core
P = 128
MT = N // P          # 32 query (m) tiles
SG = 2               # supergroups of 1024 n-columns
SGW = NH // SG       # 1024
FP32 = mybir.dt.float32
BF16 = mybir.dt.bfloat16
FP16 = mybir.dt.float16
I16 = mybir.dt.int16
NCORES = 8
NSTEPS = MT * SG     # 64

# exp(l) ~= bf16-bitcast(int16(l * 128*log2(e) + 16250.875)); the int16
# convert truncates, constant fitted for that (max rel err 3.3%)
EXP_S1 = float(np.float32(128 * 1.4426950408889634))
EXP_S2 = 16250.875
Aop = mybir.AluOpType
Afn = mybir.ActivationFunctionType

LAST_RESULTS = None  # BassKernelResults of the most recent run (for test.py)

LAG0 = 4   # o-chain lag for supergroup 0
LAG1 = 6   # o-chain lag for supergroup 1 (clears the psO slot reuse)
N_ACT = 35  # exp tiles handled by ACT (table exp); rest on DVE (bit-trick)


def _exp_pattern(n_act, n_dve):
    """Weighted round-robin ACT/DVE lane assignment for exp tiles."""
    counts = {"A": float(n_act), "D": float(n_dve)}
    total = sum(counts.values())
    acc = dict.fromkeys(counts, 0.0)
    seq = []
    for _ in range(int(total)):
        for k in counts:
            acc[k] += counts[k] / total
        pick = max(acc, key=lambda k: acc[k])
        acc[pick] -= 1.0
        seq.append(pick)
    return seq


def _body(ctx, tc, out_d, xh_d, xt_d, w3_d, wvi_d):
    nc = tc.nc
    const = ctx.enter_context(tc.tile_pool(name="const", bufs=1))
    big = ctx.enter_context(tc.tile_pool(name="big", bufs=1))
    ep = ctx.enter_context(tc.tile_pool(name="ep", bufs=10))
    obp = ctx.enter_context(tc.tile_pool(name="obp", bufs=2))
    otp = ctx.enter_context(tc.tile_pool(name="otp", bufs=2))
    spp = ctx.enter_context(tc.tile_pool(name="spp", bufs=2))
    rcp = ctx.enter_context(tc.tile_pool(name="rcp", bufs=2))
    fin = ctx.enter_context(tc.tile_pool(name="fin", bufs=2))
    psE = ctx.enter_context(tc.tile_pool(name="psE", bufs=3, space="PSUM"))
    psF = ctx.enter_context(tc.tile_pool(name="psF", bufs=1, space="PSUM"))
    psO = ctx.enter_context(tc.tile_pool(name="psO", bufs=1, space="PSUM"))

    # ---- input DMAs: xt piece0 first (critical path), weights interleaved
    xt = big.tile([P, 2, N], FP16)  # xT: [c (2 chunks of 128), m]
    w3 = const.tile([P, 3, 2, D], FP16)   # Wf/Wg/Wh stacked, [p, w, ch, d]
    wvi = const.tile([P, C + P], BF16)    # gamma*Wv (tiled 4x) | identity
    x_half = big.tile([P, MT // 2, C], FP16)

    nc.sync.dma_start(xt[:, :, 0:512], xt_d[:, :, 0:512].rearrange("c p m -> p c m"))
    nc.scalar.dma_start(w3[:], w3_d)
    for a, b in ((512, 1024), (1024, 2048), (2048, 3072), (3072, 4096)):
        nc.sync.dma_start(xt[:, :, a:b], xt_d[:, :, a:b].rearrange("c p m -> p c m"))
    nc.scalar.dma_start(wvi[:], wvi_d)
    xh_src = xh_d.rearrange("(s p) c -> p s c", p=P)
    for piece in range(4):
        nc.sync.dma_start(x_half[:, piece * 4:(piece + 1) * 4, :],
                          xh_src[:, piece * 4:(piece + 1) * 4, :])

    wvr = wvi[:, 0:C]
    ident = wvi[:, C:C + P]

    qt = big.tile([D, N], FP16)            # q: [d, m]
    kt = big.tile([D, NH], FP16)           # k: [d, n] (own half only)
    v_sb = big.tile([P, MT, D + 1], BF16)  # v: [m, d | 1]
    wsrc = big.tile([P, D], BF16, tag="wsrc")
    nc.vector.memset(wsrc[:], 0.25)
    nc.vector.memset(v_sb[:, :, D:D + 1], 1.0)

    # PE p-state warmup: tiny matmuls on a memset tile (no DMA dependency)
    warm = psE.tile([P, 64], FP32, tag="pe", name="warm")
    for _ in range(40):
        nc.tensor.matmul(
            warm[0:D, 0:D], wsrc[:], wsrc[:, 0:D],
            start=True, stop=True, skip_group_check=True,
        )
    nc.vector.tensor_copy(v_sb[0:D, 0, 0:D], warm[0:D, 0:D])  # keep it live

    # ---- projection helpers -------------------------------------------
    pool_tag = {"psE": "pe", "psF": "pf"}

    def qk_mm(kind, g, pool):
        wi = 0 if kind == "q" else 1
        pp = pool.tile([D, 512], FP32, tag=pool_tag[pool.name], name=f"p{kind}{g}")
        for ch in range(2):
            nc.tensor.matmul(
                pp[:], w3[:, wi, ch, :], xt[:, ch, g * 512:(g + 1) * 512],
                start=(ch == 0), stop=(ch == 1),
            )
        return pp

    def qk_copy(kind, g, pp, on_act):
        dst = qt if kind == "q" else kt
        if on_act:
            nc.scalar.copy(dst[:, g * 512:(g + 1) * 512], pp[:])
        else:
            nc.vector.tensor_copy(dst[:, g * 512:(g + 1) * 512], pp[:])

    def v_mm(vg, pool):
        pv = pool.tile([P, 8, D], FP32, tag=pool_tag[pool.name], name=f"pv{vg}")
        for j in range(8):
            mt = vg * 8 + j
            for ch in range(2):
                nc.tensor.matmul(
                    pv[:, j, :], xt[:, ch, mt * P:(mt + 1) * P],
                    w3[:, 2, ch, :],
                    start=(j == 0 and ch == 0), stop=(j == 7 and ch == 1),
                )
        return pv

    def v_copy(vg, pv):
        nc.vector.tensor_copy(v_sb[:, vg * 8:(vg + 1) * 8, 0:D], pv[:])

    # prologue projections (feed the first steps; use the exp pool's slots)
    q0 = qk_mm("q", 0, psE)
    qk_copy("q", 0, q0, on_act=True)
    k0 = qk_mm("k", 0, psE)
    qk_copy("k", 0, k0, on_act=False)
    k1 = qk_mm("k", 1, psE)
    qk_copy("k", 1, k1, on_act=False)
    q1 = qk_mm("q", 1, psE)
    qk_copy("q", 1, q1, on_act=True)
    v0 = v_mm(0, psE)
    v_copy(0, v0)

    # ---- deferred stage machinery -------------------------------------
    import collections
    sched = collections.defaultdict(list)

    def defer(step, fn):
        sched[step].append(fn)

    # staged projection work: (kind, idx, mm_step); copy issues next step.
    # deadlines: q mg by step 4*mg; v vg by step 8*vg+LAG0; k by 32.
    # psF has 1 buffer -> allocations spaced >= 2 steps apart; spread the
    # copies (all DVE) across the first 32 steps to keep DVE under its
    # exp-lane load.
    stages = [
        ("q", 2, 2), ("q", 3, 5), ("v", 1, 8), ("q", 4, 11), ("q", 5, 14),
        ("v", 2, 17), ("q", 6, 20), ("q", 7, 23), ("v", 3, 26),
        ("k", 2, 28), ("k", 3, 30),
    ]
    pend = {}
    for kind, idx, st in stages:
        def mk(kind, idx):
            def mm():
                if kind == "v":
                    pend[(kind, idx)] = v_mm(idx, psF)
                else:
                    pend[(kind, idx)] = qk_mm(kind, idx, psF)

            def cp():
                pp = pend.pop((kind, idx))
                if kind == "v":
                    v_copy(idx, pp)
                else:
                    qk_copy(kind, idx, pp, on_act=False)
            return mm, cp
        mm, cp = mk(kind, idx)
        defer(st, mm)
        defer(st + 1, cp)

    # ---- epilogue (per supergroup) -------------------------------------
    def emit_finals(sg, st0, pool, st_rest=None):
        ctx2 = {}
        oB = oBs[sg]

        def recs_oc():
            rec = rcp.tile([P, 8], FP32, tag="rec", name=f"rec{sg}")
            nc.vector.reciprocal(rec[:], oB[:, :, D])
            oc = obp.tile([P, 8, D], BF16, tag="ob", name=f"oc{sg}")
            nc.vector.tensor_copy(oc[:], oB[:, :, 0:D])
            ctx2["rec"] = rec
            ctx2["oc"] = oc

        def ot_mm():
            oTps = pool.tile([P, 2, P], FP32, tag=pool_tag[pool.name],
                             name=f"otp{sg}")
            oc = ctx2["oc"]
            for h2 in range(2):
                nc.tensor.matmul(oTps[:, h2, :], oc[:, h2 * 4:(h2 + 1) * 4, :],
                                 ident, start=(h2 == 0), stop=(h2 == 1))
            ctx2["otp"] = oTps

        def ot_copy():
            oT = otp.tile([P, 2, P], BF16, tag="ot", name=f"ot{sg}")
            nc.scalar.copy(oT[:], ctx2["otp"][:])
            ctx2["ot"] = oT

        def sp_mm():
            # band 3 lands at partition 96 which matmul lhsT cannot address;
            # transpose it again separately to a base-0 tile via PE
            sp_ps = pool.tile([D, 2, P], FP32, tag=pool_tag[pool.name],
                              name=f"spp{sg}")
            oc = ctx2["oc"]
            for h2 in range(2):
                nc.tensor.matmul(sp_ps[:, h2, :], oc[:, h2 * 4 + 3, :],
                                 ident, start=(h2 == 0), stop=(h2 == 1))
            ctx2["spp"] = sp_ps

        def sp_copy():
            sp = spp.tile([D, 2, P], BF16, tag="sp", name=f"sp{sg}")
            nc.vector.tensor_copy(sp[:], ctx2["spp"][:])
            ctx2["sp"] = sp

        def f_mm(h2, bd):
            def f():
                fps = pool.tile([P, C], FP32, tag=pool_tag[pool.name],
                                name=f"F{sg}_{h2}_{bd}")
                if bd < 3:
                    lhsT = ctx2["ot"][bd * D:(bd + 1) * D, h2, :]
                    rhs = wvr[bd * D:(bd + 1) * D, :]
                else:
                    lhsT = ctx2["sp"][:, h2, :]
                    rhs = wvr[0:D, :]
                nc.tensor.matmul(fps[:], lhsT, rhs, start=True, stop=True)
                ctx2[("f", h2, bd)] = fps
            return f

        def f_out(h2, bd):
            def f():
                if ("osb", h2) not in ctx2:
                    ctx2[("osb", h2)] = fin.tile([P, 4, C], FP16, tag="osb",
                                                 name=f"osb{sg}_{h2}")
                osb = ctx2[("osb", h2)]
                fps = ctx2[("f", h2, bd)]
                rec = ctx2["rec"]
                t = h2 * 4 + bd
                nt = sg * 8 + t
                on_act = (bd % 2 == 0)
                if on_act:
                    nc.scalar.activation(osb[:, bd, :], fps[:],
                                         Afn.Copy, scale=rec[:, t:t + 1])
                else:
                    nc.vector.tensor_scalar(osb[:, bd, :], fps[:],
                                            rec[:, t:t + 1], None, Aop.mult)
                # residual add: Pool (idle) for the overlapped groups; DVE
                # for the tail-critical ones (Pool's 603ns op would sit on
                # the final DMA's critical path)
                if sg == 0 or (on_act and h2 == 0):
                    nc.gpsimd.tensor_add(osb[:, bd, :], osb[:, bd, :],
                                         x_half[:, nt, :])
                else:
                    nc.vector.tensor_add(osb[:, bd, :], osb[:, bd, :],
                                         x_half[:, nt, :])
            return f

        def f_dma(h2):
            def f():
                osb = ctx2[("osb", h2)]
                base = (sg * 8 + h2 * 4) * P
                dst = out_d[base:base + 4 * P, :].rearrange(
                    "(s p) c -> p s c", p=P)
                nc.sync.dma_start(dst, osb[:])
            return f

        defer(st0, recs_oc)
        defer(st0 + 1, ot_mm)
        defer(st0 + 2, ot_copy)
        defer(st0 + 3, sp_mm)
        defer(st0 + 4, sp_copy)
        k = st0 + 5
        for h2 in range(2):
            for bd in range(4):
                defer(k, f_mm(h2, bd))
                defer(k + 1, f_out(h2, bd))
                k += 2
            defer(k, f_dma(h2))

    # ---- main loop ------------------------------------------------------
    oBs = [None, None]
    etiles = {}

    def o_step(s):
        sg, j = divmod(s, MT)
        if j == 0:
            oBs[sg] = psO.tile([P, 8, D + 1], FP32, tag="o", name=f"oB{sg}")
        oB = oBs[sg]
        et = etiles.pop(s)
        for t in range(8):
            nc.tensor.matmul(
                oB[:, t, :], et[:, t * P:(t + 1) * P], v_sb[:, j, :],
                start=(j == 0 and t == 0), stop=(j == MT - 1 and t == 7),
            )

    # o-step schedule: sg0 lags LAG0, sg1 lags LAG1 (frees psO across groups)
    o_at = {}
    for s_o in range(MT):
        o_at.setdefault(s_o + LAG0, []).append(s_o)
    for s_o in range(MT, NSTEPS):
        o_at.setdefault(s_o + LAG1, []).append(s_o)

    # phase-aware lanes: A-heavy while DVE absorbs the stage copies (steps
    # 0-31); in 32-63 DVE picks up half the sg0 epilogue scale-ops, so tilt
    # slightly less but keep ACT loaded
    pat = _exp_pattern(20, 12) + _exp_pattern(18, 14)

    def run_step(s):
        if s < NSTEPS:
            sg, mt = divmod(s, MT)
            pb = psE.tile([P, SGW], FP32, tag="pe", name=f"pb{s}")
            for hf in range(2):
                nc.tensor.matmul(
                    pb[:, hf * 512:(hf + 1) * 512],
                    qt[:, mt * P:(mt + 1) * P],
                    kt[:, sg * SGW + hf * 512:sg * SGW + (hf + 1) * 512],
                    start=True, stop=True,
                )
            et = ep.tile([P, SGW], BF16, tag="e", name=f"e{s}")
            if pat[s] == "A":
                nc.scalar.activation(et[:], pb[:], Afn.Exp)
            else:
                nc.vector.tensor_scalar(
                    et[:].bitcast(I16), pb[:],
                    EXP_S1, EXP_S2, Aop.mult, Aop.add)
            etiles[s] = et
        for s_o in o_at.get(s, ()):
            o_step(s_o)
            if s_o == MT - 1:
                emit_finals(0, s + 1, psF)
        for fn in sched.pop(s, []):
            fn()

    last = NSTEPS - 1 + LAG1
    for s in range(NSTEPS):
        run_step(s)
    for s in range(NSTEPS, last + 1):
        run_step(s)
    emit_finals(1, last + 1, psE)
    for st in sorted(sched):
        for fn in sched.pop(st):
            fn()


def build_program():
    nc = bacc.Bacc(
        "TRN2",
        target_bir_lowering=False,
        debug=False,
        enable_asserts=False,
        num_devices=NCORES,
    )
    xh_d = nc.dram_tensor("xh", [NH, C], FP16, kind="ExternalInput").ap()
    xt_d = nc.dram_tensor("xt", [2, P, N], FP16, kind="ExternalInput").ap()
    w3_d = nc.dram_tensor("W3", [P, 3, 2, D], FP16, kind="ExternalInput").ap()
    wvi_d = nc.dram_tensor("WVI", [P, C + P], BF16, kind="ExternalInput").ap()
    out_d = nc.dram_tensor("out", [NH, C], FP16, kind="ExternalOutput").ap()

    with tile.TileContext(nc) as tc:
        with ExitStack() as ctx:
            _body(ctx, tc, out_d, xh_d, xt_d, w3_d, wvi_d)
    nc.compile()
    return nc


_CACHE = {}


def _get_program():
    if "nc" not in _CACHE:
        _CACHE["nc"] = build_program()
    return _CACHE["nc"]


def make_in_maps(inputs):
    x = np.ascontiguousarray(np.asarray(inputs["x"], np.float32)).reshape(B, N, C)
    gam = np.float32(np.asarray(inputs["gamma"], np.float32).reshape(()))
    w16 = {}
    for nm in ("Wf", "Wg", "Wh"):
        w = np.asarray(inputs[nm], np.float32).astype(np.float16)  # [256, 32]
        w16[nm] = w.reshape(2, P, D)
    w3 = np.stack([w16["Wf"], w16["Wg"], w16["Wh"]])       # [3, 2, P, D]
    w3 = np.ascontiguousarray(np.transpose(w3, (2, 0, 1, 3)))  # [P, 3, 2, D]
    wv1 = (gam * np.asarray(inputs["Wv"], np.float32)).astype(ml_dtypes.bfloat16)
    wvi = np.ascontiguousarray(np.concatenate(
        [np.tile(wv1, (4, 1)), np.eye(P, dtype=ml_dtypes.bfloat16)], axis=1))

    in_maps = []
    for c in range(NCORES):
        b, h = divmod(c, 2)
        if h == 0:
            xb = x[b]
        else:
            xb = np.concatenate([x[b, NH:], x[b, :NH]], axis=0)
        xt = np.ascontiguousarray(xb.T.astype(np.float16).reshape(2, P, N))
        in_maps.append(
            {
                "xh": np.ascontiguousarray(xb[:NH].astype(np.float16)),
                "xt": xt,
                "W3": w3,
                "WVI": wvi,
            }
        )
    return in_maps


def kernel(**inputs):
    global LAST_RESULTS
    nc = _get_program()
    in_maps = make_in_maps(inputs)
    res = run_bass_kernel_spmd(nc, in_maps, core_ids=list(range(NCORES)))
    LAST_RESULTS = res
    out = np.empty((B, N, C), np.float32)
    for c in range(NCORES):
        b, h = divmod(c, 2)
        out[b, h * NH:(h + 1) * NH] = np.asarray(res.results[c]["out"], np.float32)
    return out.reshape(B, H, W, C)
